# revision 1
# baseline (speedup 1.0000x reference)
"""Multi-head attention (B=2, G=2, QLEN=KVLEN=1024, DIN=1024, H=16) on 8 TRN2
NeuronCores, pure data-parallel: core c handles (bg = c//2, q-half = c%2).

Per-core dataflow (all matmuls float32r: 11-bit mantissa at full PE speed,
fp32 PSUM accumulation; host pre-rounds inputs to the f32r grid):

  qT  [qk,  q ] = WqT.T @ xqT   (+bq per-partition on eviction)
  kT  [qk,  kv] = WkT.T @ xkvT  (+bk)
  v   [kv,  vc] = xkvT.T @ WvT  (65th col per head = ones, for softmax sums)
  per head h:
    scoresT[kv, q] = kT_h.T @ qT_h            (K=64, row-packed pairs)
    expT = exp(0.125 * scoresT)               (ACT, f32r out)
    r[65, q] = v_aug_h.T @ expT               (row 64 = softmax sums)
    bcast[128, q] = ones.T @ sums             (K=1 matmul partition-broadcast)
    recip = 1/bcast                           (DVE)
    probsT = expT * recip  -> DMA out         (f32; host un-transposes)
    rT_h = r[0:64] * recip -> resultT (f32r)
  outT [oc, q] = WoT.T @ rT (+ (bo + Wo@bv) per-partition on eviction)

Outputs per core: probsT [16, 8, 128, 512] and outT [8, 128, 512]; the host
reassembles the full (out, attn_probs) tuple.
"""
import numpy as np

import concourse.bass as bass
import concourse.mybir as mybir
import concourse.tile as tile
from concourse import bacc
from concourse.bass_utils import run_bass_kernel_spmd

B, G, QLEN, KVLEN = 2, 2, 1024, 1024
DIN = 1024
H = 16
QK, VC, OC = 1024, 1024, 1024
P = 128
QS = QLEN // 2          # q rows per core
KD = DIN // P           # contraction chunks
VH = VC // H            # 64 v-channels per head
NCORES = 8

F32R = mybir.dt.float32r
F32 = mybir.dt.float32
Exp = mybir.ActivationFunctionType.Exp

_NC_CACHE = None


def round_f32r(x: np.ndarray) -> np.ndarray:
    """Round fp32 to the fp32r grid (1-8-11, RNE) — what the PE consumes."""
    u = np.ascontiguousarray(x, dtype=np.float32).view(np.uint32).copy()
    u += 0x7FF + ((u >> 12) & np.uint32(1))
    u &= np.uint32(0xFFFFF000)
    return u.view(np.float32)


def _build():
    nc = bacc.Bacc(None, target_bir_lowering=False, debug=False)

    xqT_d = nc.declare_dram_parameter("xqT", [KD, P, QS], F32R, isOutput=False)
    xkvT_d = nc.declare_dram_parameter("xkvT", [KD, P, KVLEN], F32R, isOutput=False)
    WqT_d = nc.declare_dram_parameter("WqT", [KD, P, QK], F32R, isOutput=False)
    WkT_d = nc.declare_dram_parameter("WkT", [KD, P, QK], F32R, isOutput=False)
    WvT_d = nc.declare_dram_parameter("WvT", [KD, P, VC], F32R, isOutput=False)
    WoT_d = nc.declare_dram_parameter("WoT", [VC // P, P, OC], F32R, isOutput=False)
    bq_d = nc.declare_dram_parameter("bq", [QK // P, P, 1], F32, isOutput=False)
    bk_d = nc.declare_dram_parameter("bk", [QK // P, P, 1], F32, isOutput=False)
    bco_d = nc.declare_dram_parameter("bco", [OC // P, P, 1], F32, isOutput=False)
    probsT_d = nc.declare_dram_parameter(
        "probsT", [H, KVLEN // P, P, QS], F32, isOutput=True
    )
    outT_d = nc.declare_dram_parameter("outT", [OC // P, P, QS], F32, isOutput=True)

    with tile.TileContext(nc) as tc:
        with (
            tc.tile_pool(name="const", bufs=1) as constp,
            tc.tile_pool(name="persist", bufs=1) as persist,
            tc.tile_pool(name="wpool", bufs=4) as wpool,
        ):
            # constants
            bq_sb = constp.tile([P, QK // P], F32, name="bq_sb")
            bk_sb = constp.tile([P, QK // P], F32, name="bk_sb")
            bco_sb = constp.tile([P, OC // P], F32, name="bco_sb")
            for m in range(QK // P):
                nc.sync.dma_start(out=bq_sb[:, m:m + 1], in_=bq_d[m])
                nc.sync.dma_start(out=bk_sb[:, m:m + 1], in_=bk_d[m])
                nc.sync.dma_start(out=bco_sb[:, m:m + 1], in_=bco_d[m])
            onesf = constp.tile([P, H], F32, name="onesf")
            nc.any.memset(onesf[:], 1.0)
            ones1_r = constp.tile([1, P], F32R, name="ones1_r")
            onesf_row = constp.tile([1, P], F32, name="onesf_row")
            nc.any.memset(onesf_row[:], 1.0)
            nc.vector.tensor_copy(ones1_r[:], onesf_row[:])

            # persistent intermediates
            qT_sb = persist.tile([P, QK // P, QS], F32R, name="qT_sb")
            kT_sb = persist.tile([P, QK // P, KVLEN], F32R, name="kT_sb")
            v_sb = persist.tile([P, KVLEN // P, H, VH + 1], F32R, name="v_sb")
            rT_sb = persist.tile([P, VC // P, QS], F32R, name="rT_sb")
            for k in range(KVLEN // P):
                nc.vector.tensor_copy(v_sb[:, k, :, VH], onesf[:])

            # ---------------- Phase 1: projections ----------------
            with (
                tc.tile_pool(name="xin", bufs=1) as xin,
                tc.tile_pool(name="pp", bufs=6, space="PSUM") as pp,
            ):
                xqT_sb = xin.tile([P, KD, QS], F32R, name="xqT_sb")
                xkvT_sb = xin.tile([P, KD, KVLEN], F32R, name="xkvT_sb")
                for k in range(KD):
                    nc.sync.dma_start(out=xqT_sb[:, k, :], in_=xqT_d[k])
                    nc.sync.dma_start(out=xkvT_sb[:, k, :], in_=xkvT_d[k])

                def load_w(dram):
                    tiles = []
                    for kq in range(4):
                        wt = wpool.tile([P, 2, 1024], F32R, tag="w", name="wt")
                        for k2 in range(2):
                            nc.sync.dma_start(
                                out=wt[:, k2, :], in_=dram[kq * 2 + k2]
                            )
                        tiles.append(wt)
                    return lambda k: tiles[k // 2][:, k % 2, :]

                # Q projection: qT[m, :] = sum_k WqT[k][:, m].T @ xqT[k]
                wq = load_w(WqT_d)
                for m in range(QK // P):
                    ps = pp.tile([P, QS], F32, tag="pp", name="ps")
                    for k in range(KD):
                        nc.tensor.matmul(
                            ps[:],
                            wq(k)[:, m * P:(m + 1) * P],
                            xqT_sb[:, k, :],
                            start=(k == 0),
                            stop=(k == KD - 1),
                        )
                    nc.vector.tensor_scalar_add(
                        qT_sb[:, m, :], ps[:], bq_sb[:, m:m + 1]
                    )

                # K projection: kT[m, n*512:] = sum_k WkT[k][:, m].T @ xkvT[k, n-half]
                wk = load_w(WkT_d)
                for m in range(QK // P):
                    for n in range(2):
                        ps = pp.tile([P, QS], F32, tag="pp", name="ps")
                        for k in range(KD):
                            nc.tensor.matmul(
                                ps[:],
                                wk(k)[:, m * P:(m + 1) * P],
                                xkvT_sb[:, k, n * QS:(n + 1) * QS],
                                start=(k == 0),
                                stop=(k == KD - 1),
                            )
                        nc.vector.tensor_scalar_add(
                            kT_sb[:, m, n * QS:(n + 1) * QS], ps[:],
                            bk_sb[:, m:m + 1]
                        )

                # V projection: v[m-kv-chunk, n-half of vc] (no bias; folded
                # into the output projection's bco)
                wv = load_w(WvT_d)
                for m in range(KVLEN // P):
                    for n in range(2):
                        ps = pp.tile([P, QS], F32, tag="pp", name="ps")
                        for k in range(KD):
                            nc.tensor.matmul(
                                ps[:],
                                xkvT_sb[:, k, m * P:(m + 1) * P],
                                wv(k)[:, n * QS:(n + 1) * QS],
                                start=(k == 0),
                                stop=(k == KD - 1),
                            )
                        nc.vector.tensor_copy(
                            v_sb[:, m, 8 * n:8 * n + 8, 0:VH],
                            ps[:].rearrange("p (h c) -> p h c", h=8),
                        )

            # ---------------- Phase 2: attention heads ----------------
            with (
                tc.tile_pool(name="eTp", bufs=2) as eTp,
                tc.tile_pool(name="pTp", bufs=6) as pTp,
                tc.tile_pool(name="bcrp", bufs=2) as bcrp,
                tc.tile_pool(name="sump", bufs=2) as sump,
                tc.tile_pool(name="psc", bufs=4, space="PSUM") as psc,
                tc.tile_pool(name="pres", bufs=2, space="PSUM") as pres,
                tc.tile_pool(name="pbc", bufs=2, space="PSUM") as pbc,
            ):
                for h in range(H):
                    mq = h >> 1
                    bp = 64 * (h & 1)
                    eT = eTp.tile([P, KVLEN // P, QS], F32R, tag="eT", name="eT")
                    for j in range(KVLEN // P):
                        sc = psc.tile([P, QS], F32, tag="sc", name="sc")
                        nc.tensor.matmul(
                            sc[:],
                            kT_sb[bp:bp + 64, mq, j * P:(j + 1) * P],
                            qT_sb[bp:bp + 64, mq, :],
                            start=True,
                            stop=True,
                        )
                        nc.scalar.activation(eT[:, j, :], sc[:], Exp, scale=0.125)
                    rp = pres.tile([VH + 1, QS], F32, tag="res", name="rp")
                    for k in range(KVLEN // P):
                        nc.tensor.matmul(
                            rp[:],
                            v_sb[:, k, h, :],
                            eT[:, k, :],
                            start=(k == 0),
                            stop=(k == KVLEN // P - 1),
                        )
                    srt = sump.tile([1, QS], F32R, tag="sums", name="srt")
                    nc.vector.tensor_copy(srt[:], rp[VH:VH + 1, :])
                    bc = pbc.tile([P, QS], F32, tag="bc", name="bc")
                    nc.tensor.matmul(bc[:], ones1_r[:], srt[:], start=True, stop=True)
                    bcr = bcrp.tile([P, QS], F32, tag="bcr", name="bcr")
                    nc.vector.reciprocal(bcr[:], bc[:])
                    for j in range(KVLEN // P):
                        pT = pTp.tile([P, QS], F32, tag="pT", name="pT")
                        nc.vector.tensor_mul(
                            pT[:], eT[:, j, :].bitcast(F32), bcr[:]
                        )
                        nc.sync.dma_start(out=probsT_d[h, j], in_=pT[:])
                    nc.vector.tensor_mul(
                        rT_sb[bp:bp + 64, mq, :], rp[0:VH, :], bcr[0:64, :]
                    )

            # ---------------- Phase 3: output projection ----------------
            with (
                tc.tile_pool(name="otp", bufs=2) as otp,
                tc.tile_pool(name="ppo", bufs=2, space="PSUM") as ppo,
            ):
                wo = []
                for kq in range(4):
                    wt = wpool.tile([P, 2, OC], F32R, tag="w", name="wt")
                    for k2 in range(2):
                        nc.sync.dma_start(out=wt[:, k2, :], in_=WoT_d[kq * 2 + k2])
                    wo.append(wt)
                for m in range(OC // P):
                    po = ppo.tile([P, QS], F32, tag="po", name="po")
                    for k in range(VC // P):
                        nc.tensor.matmul(
                            po[:],
                            wo[k // 2][:, k % 2, m * P:(m + 1) * P],
                            rT_sb[:, k, :],
                            start=(k == 0),
                            stop=(k == VC // P - 1),
                        )
                    ot = otp.tile([P, QS], F32, tag="ot", name="ot")
                    nc.vector.tensor_scalar_add(ot[:], po[:], bco_sb[:, m:m + 1])
                    nc.sync.dma_start(out=outT_d[m], in_=ot[:])

    nc.compile()
    return nc


def get_nc():
    global _NC_CACHE
    if _NC_CACHE is None:
        _NC_CACHE = _build()
    return _NC_CACHE


def make_in_maps(inputs_q, inputs_kv, Wq, bq, Wk, bk, Wv, bv, Wo, bo):
    inputs_q = np.asarray(inputs_q, dtype=np.float32)
    inputs_kv = np.asarray(inputs_kv, dtype=np.float32)
    Wq = np.asarray(Wq, dtype=np.float32)
    Wk = np.asarray(Wk, dtype=np.float32)
    Wv = np.asarray(Wv, dtype=np.float32)
    Wo = np.asarray(Wo, dtype=np.float32)
    bq = np.asarray(bq, dtype=np.float32)
    bk = np.asarray(bk, dtype=np.float32)
    bv = np.asarray(bv, dtype=np.float32)
    bo = np.asarray(bo, dtype=np.float32)

    WqT = round_f32r(Wq.T).reshape(KD, P, QK)
    WkT = round_f32r(Wk.T).reshape(KD, P, QK)
    WvT = round_f32r(Wv.T).reshape(KD, P, VC)
    WoT = round_f32r(Wo.T).reshape(VC // P, P, OC)
    bq_a = bq.reshape(QK // P, P, 1)
    bk_a = bk.reshape(QK // P, P, 1)
    bco = (bo + Wo @ bv).astype(np.float32).reshape(OC // P, P, 1)

    in_maps = []
    for c in range(NCORES):
        bg, half = divmod(c, 2)
        b, g = divmod(bg, G)
        xq = inputs_q[b, g, half * QS:(half + 1) * QS, :]       # [QS, DIN]
        xkv = inputs_kv[b, g]                                   # [KVLEN, DIN]
        xqT = round_f32r(np.ascontiguousarray(xq.T)).reshape(KD, P, QS)
        xkvT = round_f32r(np.ascontiguousarray(xkv.T)).reshape(KD, P, KVLEN)
        in_maps.append({
            "xqT": xqT, "xkvT": xkvT,
            "WqT": WqT, "WkT": WkT, "WvT": WvT, "WoT": WoT,
            "bq": bq_a, "bk": bk_a, "bco": bco,
        })
    return in_maps


def assemble(results):
    out = np.empty((B, G, QLEN, OC), np.float32)
    probs = np.empty((B, G, H, QLEN, KVLEN), np.float32)
    for c in range(NCORES):
        bg, half = divmod(c, 2)
        b, g = divmod(bg, G)
        qs = slice(half * QS, (half + 1) * QS)
        outT = results[c]["outT"].reshape(OC, QS)               # [oc, q]
        out[b, g, qs, :] = outT.T
        pT = results[c]["probsT"].reshape(H, KVLEN, QS)          # [h, kv, q]
        probs[b, g, :, qs, :] = pT.transpose(0, 2, 1)
    return out, probs


def kernel(inputs_q, inputs_kv, Wq, bq, Wk, bk, Wv, bv, Wo, bo):
    nc = get_nc()
    in_maps = make_in_maps(
        inputs_q, inputs_kv, Wq, bq, Wk, bk, Wv, bv, Wo, bo
    )
    res = run_bass_kernel_spmd(nc, in_maps, core_ids=list(range(NCORES)))
    return assemble(res.results)
